# revision 39
# baseline (speedup 1.0000x reference)
"""Trainium2 Bass kernel for AdaptiveWaveletLayerSparse (GAT-style sparse
attention message passing, HOP=3) distributed over 8 NeuronCores.

Sharding: data-parallel over batch B=8 -> core i handles batch i.
Edge tables (int16, wrapped for SWDGE) are replicated to all cores.

Edges are sorted by destination on the host and padded per 32-node group
to a multiple of 128 slots. Per hop, per core:
  edge phase : dma_gather table rows by src (messages + f2[src]) and the
               f1 segment by dst; w = exp(lrelu(f1d + f2s)); scale the
               messages by w in place; segment-reduce each 128-edge block
               onto its 32-node group with a one-hot [128,32] stationary
               matmul on TensorE (accumulated in PSUM) -- no HBM scatter.
  node phase : denom -> 1/denom -> y; wavelet recurrence (precomputed
               sigmoid coefficients); build next hop's table rows (incl.
               f1/f2 matvecs); last hop projects with W via TensorE.
"""

import os
import hashlib
import numpy as np

B, N, T, C, E, HOP = 8, 4096, 24, 64, 131072, 3
TC = T * C                   # 1536
ROW = 1600                   # f32 table row: [F 1536 | f1 24 | f2 24 | pad 16]
ROWB = 1664                  # bf16 table row: [F 1536 | f1 24 | f2 24 | pad 80]
GLEN = 1600                  # gather elem (covers F+f1+f2)
F1OFF, F2OFF, PADOFF = 1536, 1560, 1584
DENOFF = 1584                # w (denominator) lives in the pad region
MCOLS = DENOFF + 24          # reduced cols: acc 1536 | f1n 24 | f2n 24 | den 24
NEG_SLOPE = 0.2
GN = 32                      # nodes per reduce group
NGRP = N // GN               # 128
NBLK = N // 128              # 32

_CACHE = {}


def _build_graph(c0, c1, c2, ch0, ch1, nhop, nblks, offs, bf16):
    import concourse.bass as bass
    import concourse.bacc as bacc
    import concourse.tile as tile
    import concourse.mybir as mybir

    f32 = mybir.dt.float32
    i16 = mybir.dt.int16
    ALU = mybir.AluOpType
    AF = mybir.ActivationFunctionType

    tdt = mybir.dt.bfloat16 if bf16 else f32
    ROWX = ROWB if bf16 else ROW
    GLENX = ROWB if bf16 else GLEN   # bf16: whole 3328B row (256B-mult)
    GDLEN = 256 if bf16 else 64

    SL = int(offs[-1])

    # Wavelet recurrence, rewritten per-hop (see reference.py):
    #   y_h = mp(in_h); in_{h+1} = s_h * y_h
    #   fp_0 = (c2 + (1-c2)*A)*x + (1-c2)*D_0*y_0
    #   fp_h = c2*fp_{h-1} + (1-c2)*A*x + (1-c2)*D_h*y_h   (h>=1)
    A_ = c0 * (2.0 * c1 - 1.0)
    D = [c1, c1 + (1.0 - c1) * (ch0 - 1.0), c1 + (1.0 - c1) * (ch1 - 1.0)]
    s = [1.0, ch0, ch1]

    nc = bacc.Bacc(None, target_bir_lowering=False)

    x_in = nc.dram_tensor("x", [N, TC], f32, kind="ExternalInput")
    srcw_d = nc.dram_tensor("srcw", [128, SL // 16], i16, kind="ExternalInput")
    dstw_d = nc.dram_tensor("dstw", [128, SL // 16], i16, kind="ExternalInput")
    sblob_d = nc.dram_tensor("sblob", [128, SL // 4], tdt, kind="ExternalInput")
    stblob_d = nc.dram_tensor("stblob", [32, SL], tdt, kind="ExternalInput")
    a1r_d = nc.dram_tensor("a1r", [128, C], f32, kind="ExternalInput")
    a2r_d = nc.dram_tensor("a2r", [128, C], f32, kind="ExternalInput")
    wpad_d = nc.dram_tensor("wpad", [128, 128], f32, kind="ExternalInput")
    br_d = nc.dram_tensor("br", [128, C], f32, kind="ExternalInput")
    id_d = nc.dram_tensor("ident", [128, 128], f32, kind="ExternalInput")
    out_d = nc.dram_tensor("out", [N, TC], f32, kind="ExternalOutput")

    tab = [nc.dram_tensor(f"tab{i}", [N + 1, ROWX], tdt) for i in range(2)]
    fptab = nc.dram_tensor("fptab", [N, TC], f32)

    with tile.TileContext(nc) as tc:
        with (
            tc.tile_pool(name="cpool", bufs=1) as cpool,
            tc.tile_pool(name="gpool", bufs=2) as gpool,
            tc.tile_pool(name="spool", bufs=3) as spool,
            tc.tile_pool(name="ipool", bufs=3) as ipool,
            tc.tile_pool(name="npool", bufs=2) as npool,
            tc.tile_pool(name="ppool", bufs=1, space="PSUM") as ppool,
        ):
            # ---- constants ----
            a1sb = cpool.tile([128, C], f32, name="a1sb")
            a2sb = cpool.tile([128, C], f32, name="a2sb")
            wpsb = cpool.tile([128, 128], f32, name="wpsb")
            brsb = cpool.tile([128, C], f32, name="brsb")
            idsb = cpool.tile([128, 128], f32, name="idsb")
            nc.sync.dma_start(a1sb[:, :], a1r_d[:, :])
            nc.sync.dma_start(a2sb[:, :], a2r_d[:, :])
            nc.sync.dma_start(wpsb[:, :], wpad_d[:, :])
            nc.sync.dma_start(brsb[:, :], br_d[:, :])
            nc.sync.dma_start(idsb[:, :], id_d[:, :])
            zr = cpool.tile([1, ROWX], tdt, name="zr")
            nc.vector.memset(zr[:, :], 0.0)
            for i in range(2):
                nc.sync.dma_start(tab[i][N:N + 1, :], zr[:, :])

            a1b = a1sb.unsqueeze(1).broadcast_to([128, T, C])
            a2b = a2sb.unsqueeze(1).broadcast_to([128, T, C])

            def build_tab_rows(tb, dst_dram, blk):
                """tb cols 0:TC already hold F; fill f1/f2/pad, DMA out."""
                tbv = tb[:, 0:TC].rearrange("p (t c) -> p t c", c=C)
                tmp = npool.tile([128, T, C], f32, name="tmp", tag="xb")
                nc.vector.tensor_tensor(tmp[:, :, :], tbv, a1b, ALU.mult)
                fr = npool.tile([128, 2, T], f32, name="fr", tag="rec")
                nc.vector.tensor_reduce(
                    fr[:, 0, :], tmp[:, :, :], mybir.AxisListType.X, ALU.add)
                tmp2 = npool.tile([128, T, C], f32, name="tmp2", tag="xb")
                nc.vector.tensor_tensor(tmp2[:, :, :], tbv, a2b, ALU.mult)
                nc.vector.tensor_reduce(
                    fr[:, 1, :], tmp2[:, :, :], mybir.AxisListType.X, ALU.add)
                nc.vector.tensor_copy(tb[:, F1OFF:F1OFF + 2 * T], fr[:, :, :])
                nc.vector.memset(tb[:, PADOFF:ROWX], 0.0)
                nc.sync.dma_start(
                    dst_dram[blk * 128:(blk + 1) * 128, :], tb[:, :])

            def project(fpt, blk):
                """out[blk] = fpt @ W + b via T -> blockdiag(W,W) -> T."""
                ost = npool.tile([128, TC], f32, name="ost", tag="tb")
                for k in range(TC // 128):
                    p1 = ppool.tile([128, 128], f32, name="p1")
                    nc.tensor.transpose(
                        p1[:, :], fpt[:, k * 128:(k + 1) * 128], idsb[:, :])
                    s1 = spool.tile([128, 128], f32, name="s1")
                    nc.scalar.copy(s1[:, :], p1[:, :])
                    p2 = ppool.tile([128, 128], f32, name="p2")
                    nc.tensor.matmul(
                        p2[:, :], wpsb[:, :], s1[:, :], start=True, stop=True)
                    s2 = spool.tile([128, 128], f32, name="s2")
                    nc.scalar.copy(s2[:, :], p2[:, :])
                    p3 = ppool.tile([128, 128], f32, name="p3")
                    nc.tensor.transpose(p3[:, :], s2[:, :], idsb[:, :])
                    ov = ost[:, k * 128:(k + 1) * 128].rearrange(
                        "p (a c) -> p a c", c=C)
                    p3v = p3.rearrange("p (a c) -> p a c", c=C)
                    bb = brsb.unsqueeze(1).broadcast_to([128, 2, C])
                    nc.vector.tensor_tensor(ov, p3v, bb, ALU.add)
                nc.sync.dma_start(
                    out_d[blk * 128:(blk + 1) * 128, :], ost[:, :])

            if nhop == 0:
                for blk in range(NBLK):
                    xb = npool.tile([128, TC], f32, name="xb", tag="xb")
                    nc.sync.dma_start(
                        xb[:, :], x_in[blk * 128:(blk + 1) * 128, :])
                    project(xb, blk)

            # ---- prologue: tab0 = [x | f1 | f2 | 0] ----
            for blk in range(NBLK if nhop > 0 else 0):
                tb = npool.tile([128, ROWX], tdt, name="tb")
                if bf16:
                    xb0 = npool.tile([128, TC], f32, name="xb0", tag="xb")
                    nc.sync.dma_start(
                        xb0[:, :], x_in[blk * 128:(blk + 1) * 128, :])
                    nc.vector.tensor_copy(tb[:, 0:TC], xb0[:, :])
                else:
                    nc.sync.dma_start(
                        tb[:, 0:TC], x_in[blk * 128:(blk + 1) * 128, :])
                build_tab_rows(tb, tab[0], blk)

            for h in range(nhop):
                tcur = tab[h % 2]
                tnext = tab[(h + 1) % 2]

                mmmode = int(os.environ.get("KERNEL_MMMODE", "0"))
                for blk in range(NBLK):
                    o = npool.tile([128, MCOLS], f32, name="o")
                    pm = None
                    if mmmode == 0:
                        pm = ppool.tile([128, 2048], f32, name="pm", tag="pm")
                    else:
                        nc.vector.memset(o[:, 0:MCOLS], 0.0)
                    # ---- edge phase: 4 groups of 32 nodes ----
                    for gg in range(4):
                        g = blk * 4 + gg
                        nb = int(nblks[g])
                        off = int(offs[g])
                        sl = nb * 128
                        ic0, icn = off // 16, sl // 16
                        si = ipool.tile([128, icn], i16, name="si", tag="si")
                        nc.sync.dma_start(si[:, :], srcw_d[:, ic0:ic0 + icn])
                        if mmmode == 3:
                            continue

                        G = gpool.tile([128, nb, GLENX], tdt, name="G",
                                       tag="G")
                        # split gathers at 1024 idxs (8 blocks) per call
                        for b0 in range(0, nb, 8):
                            bn = min(8, nb - b0)
                            ssl = bn * 128
                            isl = si[:, b0 * 8:(b0 + bn) * 8]
                            nc.gpsimd.dma_gather(
                                G[:, b0:b0 + bn, :], tcur[:, 0:GLENX], isl,
                                ssl, ssl, GLENX, elem_step=ROWX)

                        # f1[dst] by PE expand: fe[e, t] = S^T_blk @ f1g
                        f1g = ipool.tile([32, T], tdt, name="f1g", tag="f1g")
                        nc.sync.dma_start(
                            f1g[:, :],
                            bass.AP(tcur, g * GN * ROWX + F1OFF,
                                    [[ROWX, GN], [1, T]]))
                        stb = spool.tile([32, nb * 128], tdt, name="stb",
                                         tag="stb")
                        nc.sync.dma_start(
                            stb[:, :], stblob_d[:, off:off + nb * 128])
                        fe = ppool.tile([128, 512], f32, name="fe", tag="fe")
                        for j in range(nb):
                            nc.tensor.matmul(
                                fe[:, j * T:(j + 1) * T],
                                stb[:, j * 128:(j + 1) * 128],
                                f1g[:, :], start=True, stop=True)

                        z = spool.tile([128, nb, T], f32, name="z", tag="z")
                        nc.vector.tensor_tensor(
                            z[:, :, :],
                            fe[:, 0:nb * T].rearrange("p (a t) -> p a t", t=T),
                            G[:, :, F2OFF:F2OFF + T], ALU.add)
                        zl = spool.tile([128, nb, T], f32, name="zl", tag="zl")
                        nc.vector.scalar_tensor_tensor(
                            zl[:, :, :], z[:, :, :], NEG_SLOPE, z[:, :, :],
                            ALU.mult, ALU.max)
                        e = spool.tile([128, nb, T], f32, name="e", tag="e")
                        nc.scalar.activation(e[:, :, :], zl[:, :, :], AF.Exp)

                        # t-range [TH:T): expand w on ACT into a packed
                        # bf16 tile so the DVE multiply runs in 2x mode.
                        TH = T // 2
                        if bf16:
                            ew = gpool.tile([128, nb, T - TH, C], tdt,
                                            name="ew", tag="ew")
                            ebh = e[:, :, TH:T].unsqueeze(3).broadcast_to(
                                [128, nb, T - TH, C])
                            nc.scalar.copy(ew[:, :, :, :], ebh)
                            Gv2 = G[:, :, TH * C:TC].rearrange(
                                "p a (t c) -> p a t c", c=C)
                            nc.vector.tensor_tensor(
                                Gv2, Gv2, ew[:, :, :, :], ALU.mult)
                            Gv1 = G[:, :, 0:TH * C].rearrange(
                                "p a (t c) -> p a t c", c=C)
                            eb1 = e[:, :, 0:TH].unsqueeze(3).broadcast_to(
                                [128, nb, TH, C])
                            nc.vector.tensor_tensor(Gv1, Gv1, eb1, ALU.mult)
                        else:
                            Gv = G[:, :, 0:TC].rearrange(
                                "p a (t c) -> p a t c", c=C)
                            eb = e.unsqueeze(3).broadcast_to(
                                [128, nb, T, C])
                            nc.vector.tensor_tensor(Gv, Gv, eb, ALU.mult)
                        Gf = G[:, :, F1OFF:F1OFF + 2 * T].rearrange(
                            "p a (x t) -> p a x t", t=T)
                        ebf = e.unsqueeze(2).broadcast_to([128, nb, 2, T])
                        nc.vector.tensor_tensor(Gf, Gf, ebf, ALU.mult)
                        # w into the pad slot (denominator reduction)
                        nc.scalar.copy(G[:, :, DENOFF:DENOFF + T], e[:, :, :])
                        if mmmode == 2:
                            continue

                        St = spool.tile([128, nb * GN], tdt, name="St",
                                        tag="St")
                        nc.sync.dma_start(
                            St[:, :], sblob_d[:, off // 4:off // 4 + nb * GN])
                        if mmmode == 1:
                            continue
                        pms = pm[32 * gg:32 * (gg + 1), :]
                        for j in range(nb):
                            for cc0, ccn in ((0, 512), (512, 512),
                                             (1024, 512), (1536, 72)):
                                nc.tensor.matmul(
                                    pms[:, cc0:cc0 + ccn],
                                    St[:, j * GN:(j + 1) * GN],
                                    G[:, j, cc0:cc0 + ccn],
                                    start=(j == 0), stop=(j == nb - 1),
                                    tile_position=(0, 32 * gg))
                        del pms

                    if mmmode == 0:
                        nc.scalar.copy(o[:, 0:MCOLS], pm[:, 0:MCOLS])
                    # ---- node phase ----
                    den = o[:, DENOFF:DENOFF + T]
                    nc.vector.tensor_scalar_max(den, den, 1e-8)
                    rec = npool.tile([128, T], f32, name="rec")
                    nc.vector.reciprocal(rec[:, :], den)
                    yv = o[:, 0:TC].rearrange("p (t c) -> p t c", c=C)
                    recb = rec.unsqueeze(2).broadcast_to([128, T, C])
                    nc.vector.tensor_tensor(yv, yv, recb, ALU.mult)

                    # x-terms folded into one end-of-chain coefficient:
                    # fp' = sum_h c2^(last-h) * (1-c2)*D_h * y_h ; out adds
                    # alpha_x * x once at the last hop.
                    fpt = npool.tile([128, TC], f32, name="fpt")
                    if h == 0:
                        nc.vector.tensor_scalar_mul(
                            fpt[:, :], o[:, 0:TC], (1.0 - c2) * D[0])
                    else:
                        Bp = (1.0 - c2) * D[h]
                        nc.sync.dma_start(
                            fpt[:, :], fptab[blk * 128:(blk + 1) * 128, :])
                        nc.vector.scalar_tensor_tensor(
                            fpt[:, :], o[:, 0:TC], Bp / c2, fpt[:, :],
                            ALU.mult, ALU.add)
                        nc.vector.tensor_scalar_mul(fpt[:, :], fpt[:, :], c2)
                    if h == nhop - 1:
                        # alpha_x = c2^nhop + sum_{k=0}^{nhop-1} c2^k*(1-c2)*A_
                        #   (fp_0's x-coeff is c2 + (1-c2)*A_ = c2^1 + ...)
                        ax = c2 ** nhop
                        for k in range(nhop):
                            ax += (c2 ** k) * (1.0 - c2) * A_
                        xb = npool.tile([128, TC], f32, name="xb", tag="xb")
                        nc.sync.dma_start(
                            xb[:, :], x_in[blk * 128:(blk + 1) * 128, :])
                        nc.vector.scalar_tensor_tensor(
                            fpt[:, :], xb[:, :], ax, fpt[:, :],
                            ALU.mult, ALU.add)

                    if h < nhop - 1:
                        nc.sync.dma_start(
                            fptab[blk * 128:(blk + 1) * 128, :], fpt[:, :])
                        tb2 = npool.tile([128, ROWX], tdt, name="tb2",
                                         tag="tb2")
                        nc.vector.tensor_scalar_mul(
                            tb2[:, 0:TC], o[:, 0:TC], s[h])
                        # f1/f2 for the next table come from the reduce:
                        # f1_next = s * rec * sum_e w*f1_cur[src]
                        fv = o[:, F1OFF:F1OFF + 2 * T].rearrange(
                            "p (x t) -> p x t", t=T)
                        rb2 = rec.unsqueeze(1).broadcast_to([128, 2, T])
                        nc.vector.tensor_tensor(fv, fv, rb2, ALU.mult)
                        nc.vector.tensor_scalar_mul(
                            tb2[:, F1OFF:F1OFF + 2 * T],
                            o[:, F1OFF:F1OFF + 2 * T], s[h])
                        nc.vector.memset(tb2[:, PADOFF:ROWX], 0.0)
                        nc.sync.dma_start(
                            tnext[blk * 128:(blk + 1) * 128, :], tb2[:, :])
                    else:
                        project(fpt, blk)
    nc.finalize()
    return nc


def _wrap_idx(arr):
    """int array [SL] -> int16 [128, SL//16] wrapped (j -> [j%16, j//16]),
    replicated for the 8 gpsimd cores."""
    w = arr.astype(np.int16).reshape(-1, 16).T.copy()
    return np.ascontiguousarray(np.tile(w, (8, 1)))


def _prep_edges(dst, src):
    """Sort by dst, pad per 32-node group to multiples of 128 slots, build
    slot index arrays and the one-hot S blob."""
    perm = np.argsort(dst, kind="stable")
    dsts, srcs = dst[perm], src[perm]
    bounds = np.searchsorted(dsts, np.arange(0, N + 1, GN))
    cnts = np.diff(bounds)
    nblks = np.maximum(np.ceil(cnts / 128).astype(np.int64), 1)
    offs = np.concatenate([[0], np.cumsum(nblks * 128)])
    SL = int(offs[-1])
    src_slots = np.zeros(SL, np.int64)
    dst_slots = np.zeros(SL, np.int64)
    sblob = np.zeros((128, SL // 4), np.float32)
    stblob = np.zeros((32, SL), np.float32)
    for g in range(NGRP):
        lo, hi = int(bounds[g]), int(bounds[g + 1])
        cnt = hi - lo
        if cnt == 0:
            continue
        o0 = int(offs[g])
        src_slots[o0:o0 + cnt] = srcs[lo:hi]
        dst_slots[o0:o0 + cnt] = dsts[lo:hi]
        slots = np.arange(o0, o0 + cnt)
        bi = slots // 128
        r = slots % 128
        cloc = dsts[lo:hi] - g * GN
        sblob[r, bi * GN + cloc] = 1.0
        stblob[cloc, slots] = 1.0
    return (nblks, offs, _wrap_idx(src_slots), _wrap_idx(dst_slots), sblob,
            stblob)


def kernel(**inputs):
    from concourse.bass_utils import run_bass_kernel_spmd

    x = np.asarray(inputs["x"], dtype=np.float32)          # [B,N,T,C]
    edge_index = np.asarray(inputs["edge_index"])          # [2,E] int
    a = np.asarray(inputs["a"], dtype=np.float32)          # [2C,1]
    temp = np.asarray(inputs["temp"], dtype=np.float32)    # [4]
    cheb = np.asarray(inputs["cheb"], dtype=np.float32)    # [4]
    W = np.asarray(inputs["W"], dtype=np.float32)          # [C,C]
    b = np.asarray(inputs["b"], dtype=np.float32)          # [C]

    coe = 1.0 / (1.0 + np.exp(-temp.astype(np.float64)))
    chc = 1.0 / (1.0 + np.exp(-cheb.astype(np.float64)))

    dst = edge_index[0].astype(np.int64)
    src = edge_index[1].astype(np.int64)
    nblks, offs, srcw, dstw, sblob, stblob = _prep_edges(dst, src)

    nhop = int(os.environ.get("KERNEL_HOPS", str(HOP)))
    bf16 = bool(int(os.environ.get("KERNEL_BF16", "1")))
    ehash = hashlib.md5(edge_index.tobytes()).hexdigest()
    key = (round(float(coe[0]), 12), round(float(coe[1]), 12),
           round(float(coe[2]), 12), round(float(chc[0]), 12),
           round(float(chc[1]), 12), nhop, ehash, bf16,
           os.environ.get("KERNEL_MMMODE", "0"))
    if key not in _CACHE:
        _CACHE[key] = _build_graph(
            float(coe[0]), float(coe[1]), float(coe[2]),
            float(chc[0]), float(chc[1]), nhop, nblks, offs, bf16)
    nc = _CACHE[key]

    a1r = np.ascontiguousarray(np.tile(a[:C, 0], (128, 1)).astype(np.float32))
    a2r = np.ascontiguousarray(np.tile(a[C:, 0], (128, 1)).astype(np.float32))
    wpad = np.zeros((128, 128), dtype=np.float32)
    wpad[:C, :C] = W
    wpad[C:, C:] = W
    br = np.ascontiguousarray(np.tile(b, (128, 1)).astype(np.float32))
    ident = np.eye(128, dtype=np.float32)

    if bf16:
        import ml_dtypes
        sblob = sblob.astype(ml_dtypes.bfloat16)
        stblob = stblob.astype(ml_dtypes.bfloat16)
    in_maps = []
    for i in range(B):
        in_maps.append({
            "x": np.ascontiguousarray(x[i].reshape(N, TC)),
            "srcw": srcw, "dstw": dstw, "sblob": sblob, "stblob": stblob,
            "a1r": a1r, "a2r": a2r, "wpad": wpad, "br": br, "ident": ident,
        })

    trace = bool(int(os.environ.get("KERNEL_TRACE", "0")))
    res = run_bass_kernel_spmd(nc, in_maps, core_ids=list(range(B)),
                               trace=trace)
    if trace and res.exec_time_ns is not None:
        print(f"HW exec time: {res.exec_time_ns} ns")
        kernel.last_exec_time_ns = res.exec_time_ns
        kernel.last_profile = res
    out = np.stack([res.results[i]["out"] for i in range(B)])
    return out.reshape(B, N, T, C)
